# revision 20
# baseline (speedup 1.0000x reference)
"""Trainium2 Bass kernel for nn_Dep_Context_80109730005366.

Math notes (exact restructurings of the reference):
  - ctx = (q @ key) @ value is reassociated as q @ (key @ value); KV is
    [hid, c] so the huge [hw, hw] energy matrix never materializes.
  - The 1x1 conv (proj_W) and the BN scale commute with the bilinear
    upsample, so we contract KV with proj_W into a per-part [hid, hid]
    matrix (KVW) and upsample 10 channels instead of 256.
  - Coord features are input-independent; everything derived from them
    (cf, key/query constant terms) is precomputed on host as tiny matrices.

Sharding: 8 cores = 4 batches x 2 half-part groups. Core k handles batch
n = k//2 and parts {0,1,2} (k even) or {3,4,5} (k odd). The p_fea read —
the dominant HBM term, needed only to form the tiny KVW matrix — is NOT
duplicated: each core of a batch pair loads half the spatial rows,
computes a partial KVW for all 6 parts over its x-half, and a pairwise
2.4 KB AllReduce completes the sum; a per-core selection matmul then
extracts the core's own 3 parts.
"""

import numpy as np

import bass_rust
import concourse.bass as bass
import concourse.tile as tile
from concourse import mybir
from concourse.bass_utils import run_bass_kernel_spmd
from concourse.vector_clock import ScopedClock

EPS = 1e-5
N, C, H, W = 4, 256, 96, 96
HP, WP = 48, 48
HID, PARTS = 10, 6
X = HP * WP  # 2304
PPC = 3          # parts per core
PL = PPC * HID   # planes per core = 30
F32 = mybir.dt.float32

# ---------------------------------------------------------------------------
# Workaround: this container's walrus codegen rejects instructions carrying
# more than a couple of semaphore waits ("Too many sync wait commands").
# TileContext's exit path puts every outstanding wait on one Drain; spread
# them over a chain of single-wait nops instead.
# ---------------------------------------------------------------------------
_MAX_WAITS = 1


def _patched_drain_and_barrier(self, tick_clock, wait_clock):
    nc = self.nc
    drain_inst = nc.sync.drain()
    wait_clock.add_sem_waits(
        drain_inst.ins, ScopedClock({None: tick_clock.global_clock})
    )
    si = drain_inst.ins.sync_info
    waits = list(si.on_wait) if si is not None else []
    updates = list(si.on_update) if si is not None else []
    if len(waits) > _MAX_WAITS:
        drain_inst.ins.sync_info = bass_rust.SyncInfo(
            on_wait=waits[:_MAX_WAITS], on_update=updates
        )
        rest = waits[_MAX_WAITS:]
        for i in range(0, len(rest), _MAX_WAITS):
            nop = nc.sync.nop(nofuse=True, hint="split_drain_wait")
            nop.ins.sync_info = bass_rust.SyncInfo(
                on_wait=rest[i : i + _MAX_WAITS], on_update=[]
            )
    nc.all_engine_barrier()
    assert self.sems is not None
    popped = nc._tile_sem_poison_stack.pop()
    assert popped is self._sem_poison
    nc.clear_and_free_semaphores(list(self.sems.allocated().values()))
    nc.all_engine_barrier()


tile.TileContext._drain_and_barrier = _patched_drain_and_barrier

_BODY_MAX_WAITS = 1


def _split_excess_waits(nc, maxw=_BODY_MAX_WAITS):
    """Post-pass: any instruction carrying more than `maxw` semaphore waits
    gets the excess hoisted onto same-engine nops inserted right before it
    (the engine sequencer blocks on those first, preserving semantics)."""
    eng_map = {
        mybir.EngineType.SP: nc.sync,
        mybir.EngineType.PE: nc.tensor,
        mybir.EngineType.DVE: nc.vector,
        mybir.EngineType.Activation: nc.scalar,
        mybir.EngineType.Pool: nc.gpsimd,
    }

    def make_nop(engine_type, waits):
        bi = eng_map[engine_type].nop(nofuse=True, hint="wait_split")
        # pop it off the tail of the current bb; we'll splice it manually
        cur = nc.cur_bb.bb
        lst = cur.instructions
        assert lst[-1].name == bi.ins.name
        cur.instructions = lst[:-1]
        bi.ins.sync_info = bass_rust.SyncInfo(on_wait=waits, on_update=[])
        return bi.ins

    for bb in nc.m.functions[0].blocks:
        insts = bb.instructions
        out = []
        changed = False
        for inst in insts:
            si = inst.sync_info
            waits = list(si.on_wait) if si is not None else []
            if len(waits) > maxw:
                updates = list(si.on_update) if si is not None else []
                extra, keep = waits[:-maxw], waits[-maxw:]
                for j in range(0, len(extra), maxw):
                    out.append(make_nop(inst.engine, extra[j : j + maxw]))
                inst.sync_info = bass_rust.SyncInfo(on_wait=keep, on_update=updates)
                changed = True
            out.append(inst)
        if changed:
            bb.instructions = out


# ---------------------------------------------------------------------------
# Host-side constant precomputation (all tiny; heavy tensors stay on device)
# ---------------------------------------------------------------------------
def _coord_feats(hp, wp):
    xs = np.arange(wp, dtype=np.float32)
    ys = np.arange(hp, dtype=np.float32)
    xmin = xs / wp * 2 - 1
    xmax = (xs + 1) / wp * 2 - 1
    xctr = (xmin + xmax) / 2
    ymin = ys / hp * 2 - 1
    ymax = (ys + 1) / hp * 2 - 1
    yctr = (ymin + ymax) / 2
    Xb = lambda v: np.broadcast_to(v[None, :], (hp, wp))
    Yb = lambda v: np.broadcast_to(v[:, None], (hp, wp))
    ones = np.ones((hp, wp), np.float32)
    return np.stack(
        [Xb(xmin), Yb(ymin), Xb(xmax), Yb(ymax), Xb(xctr), Yb(yctr),
         ones / wp, ones / hp], axis=0,
    ).astype(np.float32)


def _interp_matrix(out_n, in_n):
    pos = np.arange(out_n, dtype=np.float32) * ((in_n - 1) / (out_n - 1))
    i0 = np.clip(np.floor(pos).astype(np.int64), 0, in_n - 1)
    i1 = np.clip(i0 + 1, 0, in_n - 1)
    w1 = (pos - i0).astype(np.float32)
    M = np.zeros((out_n, in_n), np.float32)
    for r in range(out_n):
        M[r, i0[r]] += 1 - w1[r]
        M[r, i1[r]] += w1[r]
    return M


# ---------------------------------------------------------------------------
# Device program (built once, shared SPMD across all 8 cores)
# ---------------------------------------------------------------------------
def _build_program(reps=1, debug=False, nocc=False):
    nc = bass.Bass(num_devices=8)
    dt = F32

    # per-core x-half of p_fea: input rows [xh*48, xh*48+48), xh = core % 2
    pfe = nc.dram_tensor("pfe", [C, (H // 2) * W], dt, kind="ExternalInput")
    hu3 = nc.dram_tensor("hu3", [PL, H * W], dt, kind="ExternalInput")
    # all small constants packed into one [128, 512] bank (one DMA):
    # cols 0:70 stat ch0:128, 70:140 stat ch128:256 (stat = [keyW.T | all 6
    # WpS.T]), 140:236 id96, 236:332 mT (= Mw.T = Mh.T), 332:362 qstat,
    # 362:392 bnb, 392:482 kcfT9 (9 blocks of [128, 10], this core's x-half),
    # 482:512 SEL [60, 30] part-selection matrix
    cbank = nc.dram_tensor("cbank", [128, 512], dt, kind="ExternalInput")
    qconst = nc.dram_tensor("qconst", [HID, X], dt, kind="ExternalInput")
    out3 = nc.dram_tensor("out3", [H, PL * W], dt, kind="ExternalOutput")

    HCH = 4                # h-chunks per c-tile for the p_fea x-half
    RH = 12                # input rows per chunk
    PH = RH // 2           # 6 pooled rows per chunk
    XC = PH * WP           # 288: x-chunk aligned to one pooled h-chunk
    XHALF = (HP // 2) * WP  # 1152 pooled positions per core
    NB = XHALF // 128      # 9 transpose blocks

    def alt_copy(idx, out, in_):
        # alternate psum->sbuf copies between ACT and DVE to balance engines
        if idx % 2 == 0:
            nc.scalar.copy(out, in_)
        else:
            nc.vector.tensor_copy(out, in_)

    with tile.TileContext(nc) as tc:
      for _rep in range(reps):
        with (
            tc.tile_pool(name="consts", bufs=1) as consts,
            tc.tile_pool(name="pfe_in", bufs=4) as pfe_pool,
            tc.tile_pool(name="p1", bufs=3) as p1_pool,
            tc.tile_pool(name="pf", bufs=1) as pf_pool,
            tc.tile_pool(name="hu", bufs=1) as hu_pool,
            tc.tile_pool(name="big", bufs=1) as big,
            tc.tile_pool(name="small", bufs=2) as small,
            tc.tile_pool(name="psA", bufs=2, space="PSUM") as psA,
            tc.tile_pool(name="psS", bufs=2, space="PSUM") as psS,
            tc.tile_pool(name="psCK", bufs=2, space="PSUM") as psCK,
            tc.tile_pool(name="psU", bufs=2, space="PSUM") as psU,
        ):
            # ---- constants: one packed DMA -------------------------------
            cb = consts.tile([128, 512], dt, tag="cbank", name="cbank")
            nc.sync.dma_start(cb[:], cbank[:])
            stat_sb = [cb[0:128, 0:70], cb[0:128, 70:140]]
            id96_sb = cb[0:96, 140:236]
            id70_sb = cb[0:70, 140:210]
            id10_sb = cb[0:10, 140:150]
            mwT_sb = cb[0:WP, 236:332]
            mhT_sb = cb[0:HP, 236:332]
            qstat_sb = cb[0:40, 332:362]
            bnb_sb = cb[0:H, 362:392]
            kcfT9_sb = cb[0:128, 392:482].rearrange("p (b k) -> p b k", k=10)
            sel_sb = cb[0:60, 482:512]
            huq = big.tile([40, X], dt, tag="huq")  # 0..29 pooled hu, 30..39 qconst
            nc.gpsimd.dma_start(huq[30:40, :], qconst[:])

            # ---- input DMAs: first chunks, then hu, then the rest --------
            pf_t = [
                pf_pool.tile([128, HP // 2, WP], dt, tag=f"pf{ci}", name=f"pf{ci}")
                for ci in range(2)
            ]
            hu_sb = hu_pool.tile([PL, H * W], dt, tag="hu_sb")

            chunks = []
            for hi in range(HCH):
                for ci in range(2):
                    chunk = pfe_pool.tile([128, RH, W], dt, tag="chunk", name="chunk")
                    nc.sync.dma_start(
                        chunk[:],
                        pfe[ci * 128 : (ci + 1) * 128,
                            hi * RH * W : (hi + 1) * RH * W]
                        .rearrange("c (r w) -> c r w", r=RH),
                    )
                    chunks.append((hi, ci, chunk))
                if hi == 1:
                    nc.scalar.dma_start(hu_sb[:], hu3[:])

            # ---- p_fea maxpool (DVE), one chunk at a time ----------------
            for hi, ci, chunk in chunks:
                peng = nc.vector
                p1 = p1_pool.tile([128, RH, WP], dt, tag="p1", name="p1")
                ch4 = chunk.rearrange("c r (w2 two) -> c r w2 two", two=2)
                peng.tensor_max(p1[:], ch4[:, :, :, 0], ch4[:, :, :, 1])
                p14 = p1.rearrange("c (h2 two) w -> c h2 two w", two=2)
                peng.tensor_max(
                    pf_t[ci][0:128, hi * PH : (hi + 1) * PH, :],
                    p14[:, :, 0, :],
                    p14[:, :, 1, :],
                )

            # ---- hu maxpool (DVE), 2 h-halves ----------------------------
            h1 = hu_pool.tile([PL, H, WP], dt, tag="h1")
            hu3d = hu_sb.rearrange("p (h w2 two) -> p h w2 two", h=H, two=2)
            h1p = h1.rearrange("p (h2 two) w -> p h2 two w", two=2)
            huq3d = huq[0:PL, :].rearrange("p (h w) -> p h w", h=HP)
            for half in range(2):
                peng = nc.vector
                hs = slice(half * (H // 2), (half + 1) * (H // 2))
                peng.tensor_max(
                    h1[:, hs, :], hu3d[:, hs, :, 0], hu3d[:, hs, :, 1]
                )
                hs2 = slice(half * (HP // 2), (half + 1) * (HP // 2))
                peng.tensor_max(
                    huq3d[:, hs2, :], h1p[:, hs2, 0, :], h1p[:, hs2, 1, :]
                )

            # ---- key + WpPf (stat matmul over pf x-half), chunks of 288 --
            keywp_sb = big.tile([70, XHALF], dt, tag="keywp")
            pf_f = [t.rearrange("c h w -> c (h w)") for t in pf_t]
            for xi in range(HCH):
                x0 = xi * XC
                ps = psA.tile([70, XC], dt, tag="psA", name="ps")
                nc.tensor.matmul(
                    ps[:], stat_sb[0], pf_f[0][:, x0 : x0 + XC],
                    start=True, stop=False,
                )
                nc.tensor.matmul(
                    ps[:], stat_sb[1], pf_f[1][:, x0 : x0 + XC],
                    start=False, stop=True,
                )
                alt_copy(xi, keywp_sb[:, x0 : x0 + XC], ps[:])

            # ---- transpose key|WpPf -> [x, 70] blocks; accumulate KVW ----
            # KVW partial for ALL 6 parts over this core's x-half: [10, 60]
            kvw_ps = psCK.tile([HID, 6 * HID], dt, tag="ck", name="kvw_ps")
            keywpT = big.tile([128, NB, 70], dt, tag="keywpT")
            for b in range(NB):
                tp = psS.tile([128, 70], dt, tag="pss", name="tp")
                nc.tensor.transpose(
                    tp[:], keywp_sb[:, b * 128 : (b + 1) * 128], id70_sb
                )
                # key columns get the transposed coord-const folded in
                nc.vector.tensor_add(
                    keywpT[:, b, 0:HID], tp[:, 0:HID], kcfT9_sb[:, b, :]
                )
                alt_copy(b, keywpT[:, b, HID:70], tp[:, HID:70])
            for b in range(NB):
                nc.tensor.matmul(
                    kvw_ps[:],
                    keywpT[:, b, 0:HID],
                    keywpT[:, b, HID:70],
                    start=(b == 0),
                    stop=(b == NB - 1),
                )
            kvw_half = small.tile([HID, 6 * HID], dt, tag="kvw_half")
            nc.vector.tensor_copy(kvw_half[:], kvw_ps[:])

            # ---- pairwise AllReduce completes KVW over both x-halves -----
            scr = nc.dram_tensor(f"scr{_rep}", [HID, 6 * HID], dt, kind="Internal")
            red = nc.dram_tensor(f"red{_rep}", [HID, 6 * HID], dt, kind="Internal")
            nc.sync.dma_start(scr[:], kvw_half[:])
            if nocc:
                # timing ablation: same DRAM hops, no collective
                nc.gpsimd.dma_start(red[:], scr[:])
            else:
                nc.gpsimd.collective_compute(
                    "AllReduce",
                    mybir.AluOpType.add,
                    replica_groups=[[0, 1], [2, 3], [4, 5], [6, 7]],
                    ins=[scr[:].opt()],
                    outs=[red[:].opt()],
                )
            # read back transposed: [60, 10] with the (part, out-ch) on
            # partitions, ready to be the stationary operand of the select
            kvT_sb = small.tile([6 * HID, HID], dt, tag="kvT")
            nc.sync.dma_start(kvT_sb[:], red.rearrange("a k -> k a"))
            # select this core's 3 parts: [10, 30] = kvT^T @ SEL
            own_ps = psS.tile([HID, PL], dt, tag="pss", name="own_ps")
            nc.tensor.matmul(own_ps[:], kvT_sb[:], sel_sb)
            kvw_sb = small.tile([HID, PL], dt, tag="kvw_sb")
            nc.vector.tensor_copy(kvw_sb[:], own_ps[:])
            if debug:
                dbg_half = nc.dram_tensor(
                    f"dbg_half{_rep}", [HID, 6 * HID], dt, kind="ExternalOutput")
                dbg_kvT = nc.dram_tensor(
                    f"dbg_kvT{_rep}", [6 * HID, HID], dt, kind="ExternalOutput")
                nc.scalar.dma_start(dbg_half[:], kvw_half[:])
                nc.scalar.dma_start(dbg_kvT[:], kvT_sb[:])
                dbg_kvw = nc.dram_tensor(
                    f"dbg_kvw{_rep}", [HID, PL], dt, kind="ExternalOutput")
                nc.scalar.dma_start(dbg_kvw[:], kvw_sb[:])
                dbg_bd = nc.dram_tensor(
                    f"dbg_bd{_rep}", [PL, PL], dt, kind="ExternalOutput")
                dbg_q = nc.dram_tensor(
                    f"dbg_q{_rep}", [PL, X], dt, kind="ExternalOutput")
                dbg_ctxT = nc.dram_tensor(
                    f"dbg_ctxT{_rep}", [WP, PL * HP], dt, kind="ExternalOutput")

            # block-diagonal [30, 30] version of KVW (per-part blocks);
            # three partition-shifting SBUF->SBUF DMAs on separate queues
            kvwbd = consts.tile([PL, PL], dt, tag="kvwbd")
            nc.gpsimd.memset(kvwbd[:], 0.0)
            for j, eng in zip(range(PPC), (nc.gpsimd, nc.sync, nc.scalar)):
                eng.dma_start(
                    kvwbd[j * HID : (j + 1) * HID, j * HID : (j + 1) * HID],
                    kvw_sb[:, j * HID : (j + 1) * HID],
                )

            if debug:
                nc.scalar.dma_start(dbg_bd[:], kvwbd[:])

            # ---- q_all = qstat.T @ [hu_pool; qconst] ---------------------
            q_sb = big.tile([PL, X], dt, tag="q_sb")
            for xi in range(8):
                x0 = xi * XC
                ps = psA.tile([PL, XC], dt, tag="psA", name="ps")
                nc.tensor.matmul(ps[:], qstat_sb, huq[:, x0 : x0 + XC])
                alt_copy(xi + 1, q_sb[:, x0 : x0 + XC], ps[:])
            if debug:
                nc.scalar.dma_start(dbg_q[:], q_sb[:])

            # ---- ctx, transposed, plane-major free layout ----------------
            # ctxT[w', i*48 + h'] = sum_k q[(j,k), h'w'] KVW_bd[(j,k), i]
            q3 = q_sb.rearrange("p (h w) -> p h w", h=HP)
            ctxT = big.tile([WP, PL * HP], dt, tag="ctxT")
            # view with free dims (h', i): element [w', h', i] = ctxT[w', i*48+h']
            ctxT_hi = ctxT.rearrange("w (i h) -> w h i", i=PL)
            for g in range(3):
                cps = psCK.tile([WP, 16 * PL], dt, tag="ck", name="cps")
                for hh in range(16):
                    hp_i = g * 16 + hh
                    nc.tensor.matmul(
                        cps[:, hh * PL : (hh + 1) * PL],
                        q3[:, hp_i, :],
                        kvwbd[:],
                    )
                # scatter copy psum (h-major) -> ctxT (i-major)
                cps_v = cps.rearrange("w (h i) -> w h i", h=16)
                alt_copy(g, ctxT_hi[:, g * 16 : (g + 1) * 16, :], cps_v[:])
            if debug:
                nc.scalar.dma_start(dbg_ctxT[:], ctxT[:])

            # ---- upsample stage 1: contract w' (3 big matmuls) -----------
            # a_sb[W, i*48 + h'] = sum_w' Mw[W, w'] ctxT[w', i*48+h']
            a_sb = big.tile([W, PL * HP], dt, tag="a_sb")
            for g, x0 in enumerate(range(0, PL * HP, 512)):
                xn = min(512, PL * HP - x0)
                ups = psU.tile([W, 512], dt, tag="u", name="ups")
                nc.tensor.matmul(ups[:, :xn], mwT_sb, ctxT[:, x0 : x0 + xn])
                alt_copy(g, a_sb[:, x0 : x0 + xn], ups[:, :xn])

            # ---- upsample stage 2 + BN + relu, plane groups of 6 ---------
            out_sb = big.tile([H, PL, W], dt, tag="out_sb")
            out_v = out3.rearrange("h (i w) -> h i w", w=W)
            zeros_sb = small.tile([H, W], dt, tag="zeros", name="zeros", bufs=1)
            nc.gpsimd.memset(zeros_sb[:], 0.0)
            GP = 6
            for gi in range(PL // GP):
                t2w = small.tile([HP, GP * W], dt, tag="t2w", name="t2w")
                for j3 in range(GP // 3):
                    i = gi * GP + 3 * j3
                    t2pool = psS if j3 % 2 == 0 else psCK
                    t2tag = "pss" if j3 % 2 == 0 else "ck"
                    t2 = t2pool.tile([HP, 3 * W], dt, tag=t2tag, name="t2")
                    for q_ in range(3):
                        nc.tensor.transpose(
                            t2[:, q_ * W : (q_ + 1) * W],
                            a_sb[:, (i + q_) * HP : (i + q_ + 1) * HP],
                            id96_sb,
                        )
                    alt_copy(j3, t2w[:, 3 * j3 * W : 3 * (j3 + 1) * W], t2[:])
                uppool = psU if gi % 2 == 0 else psA
                uptag = "u" if gi % 2 == 0 else "psA"
                ups2 = []
                for half in range(2):
                    up = uppool.tile([H, 3 * W], dt, tag=uptag, name="up")
                    nc.tensor.matmul(
                        up[:],
                        mhT_sb,
                        t2w[:, half * 3 * W : (half + 1) * 3 * W],
                    )
                    ups2.append(up)
                for j in range(GP):
                    i = gi * GP + j
                    up = ups2[j // 3]
                    jc = j % 3
                    if j % 2 == 0:
                        nc.scalar.activation(
                            out_sb[:, i, :], up[:, jc * W : (jc + 1) * W],
                            func=mybir.ActivationFunctionType.Relu,
                            bias=bnb_sb[:, i : i + 1],
                            scale=1.0,
                        )
                    else:
                        nc.vector.scalar_tensor_tensor(
                            out_sb[:, i, :], up[:, jc * W : (jc + 1) * W],
                            bnb_sb[:, i : i + 1], zeros_sb[:],
                            op0=mybir.AluOpType.add,
                            op1=mybir.AluOpType.max,
                        )
                nc.scalar.dma_start(
                    out_v[:, gi * GP : (gi + 1) * GP, :],
                    out_sb[:, gi * GP : (gi + 1) * GP, :],
                )

    _split_excess_waits(nc)
    return nc


_PROGRAM_CACHE = {}


def _get_program():
    if "nc" not in _PROGRAM_CACHE:
        _PROGRAM_CACHE["nc"] = _build_program()
    return _PROGRAM_CACHE["nc"]


def make_in_maps(p_fea, hu, coord_W, coord_b, query_W, query_b, key_W, key_b,
                 proj_W, bn_gamma, bn_beta, bn_mean, bn_var):
    p_fea = np.asarray(p_fea, np.float32)
    hu = np.asarray(hu, np.float32)

    # ---- host constant folding ------------------------------------------
    cf8 = _coord_feats(HP, WP).reshape(8, X)
    cf = np.asarray(coord_W, np.float32) @ cf8 + np.asarray(coord_b, np.float32)[:, None]
    kcf = np.asarray(key_W, np.float32)[:, C:] @ cf + np.asarray(key_b, np.float32)[:, None]
    qconst = (np.asarray(query_W, np.float32)[:, HID:] @ cf
              + np.asarray(query_b, np.float32)[:, None])
    Mh = _interp_matrix(H, HP)
    Mw = _interp_matrix(W, WP)
    bn_scale = np.asarray(bn_gamma, np.float32) / np.sqrt(np.asarray(bn_var, np.float32) + EPS)
    bn_bias = np.asarray(bn_beta, np.float32) - np.asarray(bn_mean, np.float32) * bn_scale
    WpS = bn_scale[:, :, None] * np.asarray(proj_W, np.float32)  # [parts, hid, c]

    qW_huT = np.asarray(query_W, np.float32)[:, :HID].T.copy()  # [10, 10] (in, out)
    keyW_cT = np.asarray(key_W, np.float32)[:, :C].T.copy()     # [256, 10]

    # stat = [keyW.T | WpS.T for all 6 parts], canonical part order (shared
    # by both cores of a pair so the AllReduce sums matching columns)
    stat = np.zeros((C, 70), np.float32)
    stat[:, 0:HID] = keyW_cT
    for p in range(PARTS):
        stat[:, HID + p * HID : HID + (p + 1) * HID] = WpS[p].T

    in_maps = []
    for core in range(8):
        n_idx = core // 2
        xh = core % 2
        pset = [0, 1, 2] if core % 2 == 0 else [3, 4, 5]

        # transposed key coord-const for this core's x-half: 9 x [128, 10]
        kcfT9 = (
            kcf[:, xh * 1152 : (xh + 1) * 1152]
            .T.reshape(9, 128, HID)
            .transpose(1, 0, 2)
            .reshape(128, 90)
        )

        qs = np.zeros((40, PL), np.float32)
        for j in range(PPC):
            qs[j * HID : (j + 1) * HID, j * HID : (j + 1) * HID] = qW_huT
            qs[PL : PL + HID, j * HID : (j + 1) * HID] = np.eye(HID, dtype=np.float32)

        bnb = np.zeros((H, PL), np.float32)
        for j, p in enumerate(pset):
            bnb[:, j * HID : (j + 1) * HID] = bn_bias[p][None, :]

        sel = np.zeros((60, PL), np.float32)
        for j, p in enumerate(pset):
            sel[p * HID : (p + 1) * HID, j * HID : (j + 1) * HID] = np.eye(
                HID, dtype=np.float32
            )

        cbank = np.zeros((128, 512), np.float32)
        cbank[0:128, 0:70] = stat[0:128]
        cbank[0:128, 70:140] = stat[128:256]
        cbank[0:96, 140:236] = np.eye(96, dtype=np.float32)
        cbank[0:WP, 236:332] = Mw.T
        cbank[0:40, 332:362] = qs
        cbank[0:H, 362:392] = bnb
        cbank[0:128, 392:482] = kcfT9
        cbank[0:60, 482:512] = sel
        in_maps.append({
            "pfe": np.ascontiguousarray(
                p_fea[n_idx, :, xh * 48 : (xh + 1) * 48, :].reshape(C, (H // 2) * W)
            ),
            "hu3": np.ascontiguousarray(hu[pset, n_idx].reshape(PL, H * W)),
            "cbank": cbank,
            "qconst": np.ascontiguousarray(qconst),
        })
    return in_maps


def assemble_out(results):
    out = np.empty((PARTS, N, HID, H, W), np.float32)
    for core in range(8):
        n_idx = core // 2
        pset = [0, 1, 2] if core % 2 == 0 else [3, 4, 5]
        r = results[core]["out3"].reshape(H, PPC, HID, W).transpose(1, 2, 0, 3)
        out[pset, n_idx] = r
    return out


def kernel(**inputs):
    in_maps = make_in_maps(**inputs)
    nc = _get_program()
    res = run_bass_kernel_spmd(nc, in_maps, core_ids=list(range(8)))
    return assemble_out(res.results)

